# revision 8
# baseline (speedup 1.0000x reference)
"""CondConv (dense_cnn) Trainium2 kernel — 8-core data-parallel over batch.

Reference computation (per batch of 16):
  routing = sigmoid(GAP(x) @ fc_w.T + fc_b)                  (16, 8)
  comb_w  = einsum('bk,koihw->boihw', routing, expert_w)     per-sample 3x3 kernels
  out     = per-sample conv2d(x, comb_w, pad=1) + comb_b
  out     = BatchNorm2d(train stats over (b,h,w)) -> LeakyReLU(0.1)

Sharding: batch split 2 samples/core across 8 cores. Each core computes its
own routing + combined kernels (DVE fused multiply-accumulate over the 8
experts) and runs the conv as shift-GEMM (18 accumulating float32r matmuls
per [128 c_out, 512 px] PSUM tile: 2 c_in halves x 9 taps). BN batch stats
(sum, sum of squares per channel) are all-reduced across the 8 cores per
c_out tile of 128 channels, so normalize+LeakyReLU of tile t overlaps the
conv of tile t+1.
"""

import sys

for _p in ("/opt/trn_rl_repo", "/root/.axon_site/_ro/trn_rl_repo"):
    if _p not in sys.path:
        sys.path.insert(0, _p)

import numpy as np

import concourse.bass as bass  # noqa: F401  (engine types)
import concourse.tile as tile
from concourse import bacc, mybir
from concourse.bass_utils import run_bass_kernel_spmd

N_CORES = 8
B, CIN, H, W = 16, 256, 64, 64
COUT, KH, NEXP = 512, 3, 8
SPC = B // N_CORES            # samples per core
HP = H + 2                    # padded image side
FREE_PAD = HP * HP            # 4356
NCT = COUT // 128             # c_out tiles (4)
NHT = 8                       # hw tiles per image: 8 rows of 8 image-rows
EPS = 1e-5
NORM = 1.0 / (B * H * W)      # BN divisor (65536)

F32 = mybir.dt.float32
F32R = mybir.dt.float32r
BF16 = mybir.dt.bfloat16
ADD = mybir.AluOpType.add
MULT = mybir.AluOpType.mult
SUB = mybir.AluOpType.subtract
MAX = mybir.AluOpType.max
AX = mybir.AxisListType.X
AF = mybir.ActivationFunctionType


def _emit(tc):
    nc = tc.nc
    xp_e = nc.dram_tensor("xp", [SPC, 2, 128, FREE_PAD], F32, kind="ExternalInput")
    ew_e = nc.dram_tensor("ew", [NCT, 2, NEXP, 128, 9 * 128], F32, kind="ExternalInput")
    eb_e = nc.dram_tensor("eb", [NEXP, COUT], F32, kind="ExternalInput")
    fcw_e = nc.dram_tensor("fcw", [2, 128, NEXP], F32, kind="ExternalInput")
    fcb_e = nc.dram_tensor("fcb", [NEXP, 1], F32, kind="ExternalInput")
    gt_e = nc.dram_tensor("gt", [128, NCT], F32, kind="ExternalInput")
    bt_e = nc.dram_tensor("bt", [128, NCT], F32, kind="ExternalInput")
    out_e = nc.dram_tensor("out", [SPC, COUT, H * W], F32, kind="ExternalOutput")

    with (
        tc.tile_pool(name="persist", bufs=1) as pp,
        tc.tile_pool(name="xpool", bufs=1) as xpool,
        tc.tile_pool(name="estream", bufs=6) as epool,
        tc.tile_pool(name="comb", bufs=8) as cpool,
        tc.tile_pool(name="stage", bufs=2) as stpool,
        tc.tile_pool(name="stats", bufs=2) as statp,
        tc.tile_pool(name="bn", bufs=2) as bnp,
        tc.tile_pool(name="work", bufs=3) as wkp,
        tc.tile_pool(name="cpsum", bufs=4, space="PSUM") as ppool,
        tc.tile_pool(name="spsum", bufs=1, space="PSUM") as psmall,
        tc.tile_pool(name="dram", bufs=4, space="DRAM") as dpool,
    ):
        # ---- load persistent data ----
        xp_sb = {}
        for s in range(SPC):
            for c in range(2):
                t_ = xpool.tile([128, FREE_PAD], F32R, tag=f"xp_{s}_{c}")
                nc.sync.dma_start(t_[:], xp_e.ap()[s, c].bitcast(F32R))
                xp_sb[(s, c)] = t_
        fcw_sb = pp.tile([128, 2 * NEXP], F32, tag="fcw")
        for c in range(2):
            nc.sync.dma_start(fcw_sb[:, c * NEXP:(c + 1) * NEXP], fcw_e.ap()[c])
        fcb_sb = pp.tile([NEXP, 1], F32, tag="fcb")
        nc.sync.dma_start(fcb_sb[:], fcb_e.ap())
        eb_sb = pp.tile([NEXP, COUT], F32, tag="eb")
        nc.sync.dma_start(eb_sb[:], eb_e.ap())
        gt_sb = pp.tile([128, NCT], F32, tag="gt")
        nc.sync.dma_start(gt_sb[:], gt_e.ap())
        bt_sb = pp.tile([128, NCT], F32, tag="bt")
        nc.sync.dma_start(bt_sb[:], bt_e.ap())
        eps_sb = pp.tile([128, 1], F32, tag="eps")
        nc.gpsimd.memset(eps_sb[:], EPS)

        # ---- routing: GAP -> linear -> sigmoid ----
        # fold the 1/(H*W) GAP divisor into fc_w once
        nc.scalar.mul(fcw_sb[:], fcw_sb[:], 1.0 / (H * W))
        pooled = pp.tile([128, 2 * SPC], F32, tag="pooled")
        for s in range(SPC):
            for c in range(2):
                col = c * SPC + s
                nc.vector.tensor_reduce(
                    pooled[:, col:col + 1], xp_sb[(s, c)][:].bitcast(F32),
                    axis=AX, op=ADD,
                )
        rpsum = psmall.tile([NEXP, SPC], F32, tag="rpsum")
        for c in range(2):
            nc.tensor.matmul(
                rpsum[:],
                fcw_sb[:, c * NEXP:(c + 1) * NEXP],
                pooled[:, c * SPC:(c + 1) * SPC],
                start=(c == 0),
                stop=(c == 1),
            )
        routing_sb = pp.tile([NEXP, SPC], F32, tag="routing")
        nc.scalar.activation(routing_sb[:], rpsum[:], AF.Sigmoid, bias=fcb_sb[:])

        # broadcast routing to all 128 partitions: [8,2] -> dram -> [1,16] -> [128,16]
        r_scr = dpool.tile([1, NEXP * SPC], F32, tag="r_scr")
        r_view = r_scr[:].rearrange("o (a b) -> (o a) b", a=NEXP, b=SPC)
        nc.sync.dma_start(r_view, routing_sb[:])
        r_row = pp.tile([1, NEXP * SPC], F32, tag="r_row")
        nc.sync.dma_start(r_row[:], r_scr[:])
        rbc = pp.tile([128, NEXP * SPC], F32, tag="rbc")
        nc.gpsimd.partition_broadcast(rbc[:], r_row[:])

        # combined per-sample bias: comb_b[c, s] = sum_k eb[k, c] * r[k, s]
        combb = pp.tile([128, NCT * SPC], F32, tag="combb")
        for t in range(NCT):
            bpsum = psmall.tile([128, SPC], F32, tag="bpsum")
            nc.tensor.matmul(
                bpsum[:], eb_sb[:, t * 128:(t + 1) * 128], routing_sb[:],
                start=True, stop=True,
            )
            nc.scalar.copy(combb[:, t * SPC:(t + 1) * SPC], bpsum[:])

        # ---- main loop over c_out tiles ----
        for t in range(NCT):
            # combine expert kernels for this c_out tile (DVE FMA chain)
            comb = {}
            for c in range(2):
                for k in range(NEXP):
                    es = epool.tile([128, 9 * 128], F32, tag="es")
                    nc.sync.dma_start(es[:], ew_e.ap()[t, c, k])
                    for s in range(SPC):
                        sc = rbc[:, k * SPC + s:k * SPC + s + 1]
                        if k == 0:
                            cb = cpool.tile([128, 9 * 128], F32R, tag="comb")
                            comb[(s, c)] = cb
                            nc.vector.tensor_scalar(cb[:], es[:], sc, None, MULT)
                        else:
                            nc.vector.scalar_tensor_tensor(
                                comb[(s, c)][:], es[:], sc, comb[(s, c)][:],
                                MULT, ADD,
                            )

            # conv + epilogue for this tile
            stage = stpool.tile([128, SPC * H * W], BF16, tag="stage")
            sums = statp.tile([128, SPC * NHT], F32, tag="sums")
            sumsq = statp.tile([128, SPC * NHT], F32, tag="sumsq")
            for s in range(SPC):
                for h in range(NHT):
                    ps = ppool.tile([128, 512], F32, tag="cps")
                    first = True
                    for c in range(2):
                        xv = xp_sb[(s, c)][:].rearrange("p (i j) -> p i j", j=HP)
                        for tap in range(9):
                            dy, dx = divmod(tap, 3)
                            rhs = xv[:, h * 8 + dy:h * 8 + dy + 8, dx:dx + W]
                            lhsT = comb[(s, c)][:, tap * 128:(tap + 1) * 128]
                            nc.tensor.matmul(
                                ps[:], lhsT, rhs,
                                start=first, stop=(c == 1 and tap == 8),
                            )
                            first = False
                    idx = s * NHT + h
                    nc.scalar.activation(
                        stage[:, idx * 512:(idx + 1) * 512], ps[:], AF.Identity,
                        bias=combb[:, t * SPC + s:t * SPC + s + 1],
                        accum_out=sums[:, idx:idx + 1],
                    )
                    sq = wkp.tile([128, 512], F32, tag="sq")
                    nc.scalar.activation(
                        sq[:], stage[:, idx * 512:(idx + 1) * 512], AF.Square,
                        accum_out=sumsq[:, idx:idx + 1],
                    )

            # ---- BN stats all-reduce for channels [t*128, (t+1)*128) ----
            stats_sb = bnp.tile([128, 2], F32, tag="stats_sb")
            nc.vector.tensor_reduce(stats_sb[:, 0:1], sums[:], axis=AX, op=ADD)
            nc.vector.tensor_reduce(stats_sb[:, 1:2], sumsq[:], axis=AX, op=ADD)
            st_in = dpool.tile([128, 2], F32, tag="st_in")
            st_out = dpool.tile([128, 2], F32, tag="st_out")
            nc.sync.dma_start(st_in[:], stats_sb[:])
            nc.gpsimd.collective_compute(
                "AllReduce", ADD,
                replica_groups=[list(range(N_CORES))],
                ins=[st_in.opt()],
                outs=[st_out.opt()],
            )
            gstats = bnp.tile([128, 2], F32, tag="gstats")
            nc.sync.dma_start(gstats[:], st_out[:])

            mean = bnp.tile([128, 1], F32, tag="mean")
            nc.scalar.mul(mean[:], gstats[:, 0:1], NORM)
            ex2 = bnp.tile([128, 1], F32, tag="ex2")
            nc.scalar.mul(ex2[:], gstats[:, 1:2], NORM)
            m2 = bnp.tile([128, 1], F32, tag="m2")
            nc.scalar.activation(m2[:], mean[:], AF.Square)
            var = bnp.tile([128, 1], F32, tag="var")
            nc.vector.tensor_tensor(var[:], ex2[:], m2[:], SUB)
            std = bnp.tile([128, 1], F32, tag="std")
            nc.scalar.activation(std[:], var[:], AF.Sqrt, bias=eps_sb[:])
            rstd = bnp.tile([128, 1], F32, tag="rstd")
            nc.vector.reciprocal(rstd[:], std[:])
            scale = bnp.tile([128, 1], F32, tag="scale")
            nc.vector.tensor_tensor(scale[:], gt_sb[:, t:t + 1], rstd[:], MULT)
            nscale = bnp.tile([128, 1], F32, tag="nscale")
            nc.scalar.mul(nscale[:], scale[:], -1.0)
            shift = bnp.tile([128, 1], F32, tag="shift")
            nc.vector.scalar_tensor_tensor(
                shift[:], mean[:], nscale[:], bt_sb[:, t:t + 1], MULT, ADD
            )

            # ---- normalize + LeakyReLU + store ----
            for s in range(SPC):
                for h in range(NHT):
                    idx = s * NHT + h
                    z = wkp.tile([128, 512], F32, tag="z")
                    nc.scalar.activation(
                        z[:], stage[:, idx * 512:(idx + 1) * 512], AF.Identity,
                        bias=shift[:], scale=scale[:],
                    )
                    f = wkp.tile([128, 512], F32, tag="f")
                    nc.vector.scalar_tensor_tensor(f[:], z[:], 0.1, z[:], MULT, MAX)
                    nc.sync.dma_start(
                        out_e.ap()[s, t * 128:(t + 1) * 128, h * 512:(h + 1) * 512],
                        f[:],
                    )


_NC_CACHE = []


def _build():
    if _NC_CACHE:
        return _NC_CACHE[0]
    nc = bacc.Bacc("TRN2", target_bir_lowering=False, debug=False,
                   num_devices=N_CORES)
    with tile.TileContext(nc) as tc:
        _emit(tc)
    nc.compile()
    _NC_CACHE.append(nc)
    return nc


def make_in_maps(x, expert_weight, expert_bias, fc_w, fc_b, bn_gamma, bn_beta):
    x = np.asarray(x, np.float32)
    ew = np.asarray(expert_weight, np.float32)
    # padded input, split into c_in halves: xp[b, c, p, (i, j)]
    xp = np.zeros((B, 2, 128, HP, HP), np.float32)
    xp[:, :, :, 1:H + 1, 1:W + 1] = x.reshape(B, 2, 128, H, W)
    xp = np.ascontiguousarray(xp.reshape(B, 2, 128, FREE_PAD))
    # expert weights in conv-lhsT layout: [t, c, k, p=ci, tap*128 + m=co]
    e = ew.reshape(NEXP, NCT, 128, 2, 128, KH, KH)
    ewt = np.ascontiguousarray(
        e.transpose(1, 3, 0, 4, 5, 6, 2).reshape(NCT, 2, NEXP, 128, 9 * 128)
    )
    fcwt = np.ascontiguousarray(
        np.asarray(fc_w, np.float32).T.reshape(2, 128, NEXP)
    )
    fcb = np.ascontiguousarray(np.asarray(fc_b, np.float32).reshape(NEXP, 1))
    eb = np.ascontiguousarray(np.asarray(expert_bias, np.float32))
    gt = np.ascontiguousarray(np.asarray(bn_gamma, np.float32).reshape(NCT, 128).T)
    bt = np.ascontiguousarray(np.asarray(bn_beta, np.float32).reshape(NCT, 128).T)
    return [
        {
            "xp": np.ascontiguousarray(xp[j * SPC:(j + 1) * SPC]),
            "ew": ewt,
            "eb": eb,
            "fcw": fcwt,
            "fcb": fcb,
            "gt": gt,
            "bt": bt,
        }
        for j in range(N_CORES)
    ]


def kernel(x, expert_weight, expert_bias, fc_w, fc_b, bn_gamma, bn_beta):
    nc = _build()
    in_maps = make_in_maps(x, expert_weight, expert_bias, fc_w, fc_b,
                           bn_gamma, bn_beta)
    res = run_bass_kernel_spmd(nc, in_maps, list(range(N_CORES)))
    return np.concatenate(
        [r["out"].reshape(SPC, COUT, H, W) for r in res.results], axis=0
    ).astype(np.float32)


# revision 16
# speedup vs baseline: 302.5797x; 302.5797x over previous
"""CondConv (dense_cnn) Trainium2 kernel — 8-core data-parallel over batch.

Reference computation (per batch of 16):
  routing = sigmoid(GAP(x) @ fc_w.T + fc_b)                  (16, 8)
  comb_w  = einsum('bk,koihw->boihw', routing, expert_w)     per-sample 3x3 kernels
  out     = per-sample conv2d(x, comb_w, pad=1) + comb_b
  out     = BatchNorm2d(train stats over (b,h,w)) -> LeakyReLU(0.1)

Sharding: batch split 2 samples/core across 8 cores. Each core computes its
own routing + combined kernels (DVE fused multiply-accumulate over the 8
experts) and runs the conv as shift-GEMM (18 accumulating float32r matmuls
per [128 c_out, 512 px] PSUM tile: 2 c_in halves x 9 taps). BN batch stats
(sum, sum of squares per channel) are all-reduced across the 8 cores per
c_out tile of 128 channels, so normalize+LeakyReLU of tile t overlaps the
conv of tile t+1.
"""

import sys

for _p in ("/opt/trn_rl_repo", "/root/.axon_site/_ro/trn_rl_repo"):
    if _p not in sys.path:
        sys.path.insert(0, _p)

import numpy as np

import concourse.bass as bass  # noqa: F401  (engine types)
import concourse.tile as tile
from concourse import bacc, mybir
from concourse.bass_utils import run_bass_kernel_spmd

N_CORES = 8
B, CIN, H, W = 16, 256, 64, 64
COUT, KH, NEXP = 512, 3, 8
SPC = B // N_CORES            # samples per core
HP = H + 2                    # padded image side
FREE_PAD = HP * HP            # 4356
NCT = COUT // 128             # c_out tiles (4)
NHT = 8                       # hw tiles per image: 8 rows of 8 image-rows
EPS = 1e-5
NORM = 1.0 / (B * H * W)      # BN divisor (65536)

F32 = mybir.dt.float32
F32R = mybir.dt.float32r
BF16 = mybir.dt.bfloat16
ADD = mybir.AluOpType.add
MULT = mybir.AluOpType.mult
SUB = mybir.AluOpType.subtract
MAX = mybir.AluOpType.max
AX = mybir.AxisListType.X
AF = mybir.ActivationFunctionType


def _emit(tc, reps=1, use_collective=True):
    nc = tc.nc
    xp_e = nc.dram_tensor("xp", [SPC, 2, 128, FREE_PAD], F32, kind="ExternalInput")
    ew_e = nc.dram_tensor("ew", [NCT, 2, NEXP, 128, 9 * 128], F32, kind="ExternalInput")
    eb_e = nc.dram_tensor("eb", [NEXP, COUT], F32, kind="ExternalInput")
    fcw_e = nc.dram_tensor("fcw", [2, 128, NEXP], F32, kind="ExternalInput")
    fcb_e = nc.dram_tensor("fcb", [NEXP, 1], F32, kind="ExternalInput")
    gt_e = nc.dram_tensor("gt", [128, NCT], F32, kind="ExternalInput")
    bt_e = nc.dram_tensor("bt", [128, NCT], F32, kind="ExternalInput")
    out_e = nc.dram_tensor("out", [SPC, COUT, H * W], F32, kind="ExternalOutput")

    with (
        tc.tile_pool(name="persist", bufs=1) as pp,
        tc.tile_pool(name="xpool", bufs=1) as xpool,
        tc.tile_pool(name="estream", bufs=6) as epool,
        tc.tile_pool(name="comb", bufs=8) as cpool,
        tc.tile_pool(name="stage", bufs=2) as stpool,
        tc.tile_pool(name="stats", bufs=2) as statp,
        tc.tile_pool(name="bn", bufs=2) as bnp,
        tc.tile_pool(name="work", bufs=3) as wkp,
        tc.tile_pool(name="cpsum", bufs=4, space="PSUM") as ppool,
        tc.tile_pool(name="spsum", bufs=1, space="PSUM") as psmall,
        tc.tile_pool(name="dram", bufs=4, space="DRAM") as dpool,
    ):
        # ---- load persistent data ----
        xp_sb = {}
        for s in range(SPC):
            for c in range(2):
                t_ = xpool.tile([128, FREE_PAD], F32R, tag=f"xp_{s}_{c}")
                nc.sync.dma_start(t_[:], xp_e.ap()[s, c].bitcast(F32R))
                xp_sb[(s, c)] = t_
        fcw_sb = pp.tile([128, 2 * NEXP], F32, tag="fcw")
        for c in range(2):
            nc.sync.dma_start(fcw_sb[:, c * NEXP:(c + 1) * NEXP], fcw_e.ap()[c])
        fcb_sb = pp.tile([NEXP, 1], F32, tag="fcb")
        nc.sync.dma_start(fcb_sb[:], fcb_e.ap())
        eb_sb = pp.tile([NEXP, COUT], F32, tag="eb")
        nc.sync.dma_start(eb_sb[:], eb_e.ap())
        gt_sb = pp.tile([128, NCT], F32, tag="gt")
        nc.sync.dma_start(gt_sb[:], gt_e.ap())
        bt_sb = pp.tile([128, NCT], F32, tag="bt")
        nc.sync.dma_start(bt_sb[:], bt_e.ap())
        eps_sb = pp.tile([128, 1], F32, tag="eps")
        nc.gpsimd.memset(eps_sb[:], EPS)

        # ---- routing: GAP -> linear -> sigmoid ----
        # fold the 1/(H*W) GAP divisor into fc_w once
        nc.scalar.mul(fcw_sb[:], fcw_sb[:], 1.0 / (H * W))

        def _main_body():
            _emit_main(tc, nc, pp, epool, cpool, stpool, statp, bnp, wkp,
                       ppool, psmall, dpool, xp_sb, fcw_sb, fcb_sb, eb_sb,
                       gt_sb, bt_sb, eps_sb, ew_e, out_e, use_collective)

        if reps == 1:
            _main_body()
        else:
            with tc.For_i(0, reps, 1):
                _main_body()


def _emit_main(tc, nc, pp, epool, cpool, stpool, statp, bnp, wkp, ppool,
               psmall, dpool, xp_sb, fcw_sb, fcb_sb, eb_sb, gt_sb, bt_sb,
               eps_sb, ew_e, out_e, use_collective=True):
    if True:
        pooled = pp.tile([128, 2 * SPC], F32, tag="pooled")
        for s in range(SPC):
            for c in range(2):
                col = c * SPC + s
                nc.vector.tensor_reduce(
                    pooled[:, col:col + 1], xp_sb[(s, c)][:].bitcast(F32),
                    axis=AX, op=ADD,
                )
        rpsum = psmall.tile([NEXP, SPC], F32, tag="rpsum")
        for c in range(2):
            nc.tensor.matmul(
                rpsum[:],
                fcw_sb[:, c * NEXP:(c + 1) * NEXP],
                pooled[:, c * SPC:(c + 1) * SPC],
                start=(c == 0),
                stop=(c == 1),
            )
        routing_sb = pp.tile([NEXP, SPC], F32, tag="routing")
        nc.scalar.activation(routing_sb[:], rpsum[:], AF.Sigmoid, bias=fcb_sb[:])

        # broadcast routing to all 128 partitions: [8,2] -> dram -> [1,16] -> [128,16]
        r_scr = dpool.tile([1, NEXP * SPC], F32, tag="r_scr")
        r_view = r_scr[:].rearrange("o (a b) -> (o a) b", a=NEXP, b=SPC)
        nc.sync.dma_start(r_view, routing_sb[:])
        r_row = pp.tile([1, NEXP * SPC], F32, tag="r_row")
        nc.sync.dma_start(r_row[:], r_scr[:])
        rbc = pp.tile([128, NEXP * SPC], F32, tag="rbc")
        nc.gpsimd.partition_broadcast(rbc[:], r_row[:])

        # combined per-sample bias: comb_b[c, s] = sum_k eb[k, c] * r[k, s]
        combb = pp.tile([128, NCT * SPC], F32, tag="combb")
        for t in range(NCT):
            bpsum = psmall.tile([128, SPC], F32, tag="bpsum")
            nc.tensor.matmul(
                bpsum[:], eb_sb[:, t * 128:(t + 1) * 128], routing_sb[:],
                start=True, stop=True,
            )
            nc.scalar.copy(combb[:, t * SPC:(t + 1) * SPC], bpsum[:])

        # ---- main loop over c_out tiles ----
        for t in range(NCT):
            # combine expert kernels for this c_out tile (DVE FMA chain)
            comb = {}
            for c in range(2):
                for k in range(NEXP):
                    es = epool.tile([128, 9 * 128], F32, tag="es")
                    nc.sync.dma_start(es[:], ew_e.ap()[t, c, k])
                    for s in range(SPC):
                        sc = rbc[:, k * SPC + s:k * SPC + s + 1]
                        if k == 0:
                            cb = cpool.tile([128, 9 * 128], F32R, tag="comb")
                            comb[(s, c)] = cb
                            nc.vector.tensor_scalar(cb[:], es[:], sc, None, MULT)
                        else:
                            nc.vector.scalar_tensor_tensor(
                                comb[(s, c)][:], es[:], sc, comb[(s, c)][:],
                                MULT, ADD,
                            )

            # conv + epilogue for this tile
            stage = stpool.tile([128, SPC * H * W], BF16, tag="stage")
            sums = statp.tile([128, SPC * NHT], F32, tag="sums")
            sumsq = statp.tile([128, SPC * NHT], F32, tag="sumsq")
            for s in range(SPC):
                for h in range(NHT):
                    ps = ppool.tile([128, 512], F32, tag="cps")
                    first = True
                    for c in range(2):
                        xv = xp_sb[(s, c)][:].rearrange("p (i j) -> p i j", j=HP)
                        for tap in range(9):
                            dy, dx = divmod(tap, 3)
                            rhs = xv[:, h * 8 + dy:h * 8 + dy + 8, dx:dx + W]
                            lhsT = comb[(s, c)][:, tap * 128:(tap + 1) * 128]
                            nc.tensor.matmul(
                                ps[:], lhsT, rhs,
                                start=first, stop=(c == 1 and tap == 8),
                            )
                            first = False
                    idx = s * NHT + h
                    nc.scalar.activation(
                        stage[:, idx * 512:(idx + 1) * 512], ps[:], AF.Identity,
                        bias=combb[:, t * SPC + s:t * SPC + s + 1],
                        accum_out=sums[:, idx:idx + 1],
                    )
                    sq = wkp.tile([128, 512], F32, tag="sq")
                    nc.scalar.activation(
                        sq[:], stage[:, idx * 512:(idx + 1) * 512], AF.Square,
                        accum_out=sumsq[:, idx:idx + 1],
                    )

            # ---- BN stats all-reduce for channels [t*128, (t+1)*128) ----
            stats_sb = bnp.tile([128, 2], F32, tag="stats_sb")
            nc.vector.tensor_reduce(stats_sb[:, 0:1], sums[:], axis=AX, op=ADD)
            nc.vector.tensor_reduce(stats_sb[:, 1:2], sumsq[:], axis=AX, op=ADD)
            st_in = dpool.tile([128, 2], F32, tag="st_in")
            st_out = dpool.tile([128, 2], F32, tag="st_out")
            nc.sync.dma_start(st_in[:], stats_sb[:])
            if use_collective:
                nc.gpsimd.collective_compute(
                    "AllReduce", ADD,
                    replica_groups=[list(range(N_CORES))],
                    ins=[st_in.opt()],
                    outs=[st_out.opt()],
                )
            else:
                nc.sync.dma_start(st_out[:], st_in[:])
            gstats = bnp.tile([128, 2], F32, tag="gstats")
            nc.sync.dma_start(gstats[:], st_out[:])

            mean = bnp.tile([128, 1], F32, tag="mean")
            nc.scalar.mul(mean[:], gstats[:, 0:1], NORM)
            ex2 = bnp.tile([128, 1], F32, tag="ex2")
            nc.scalar.mul(ex2[:], gstats[:, 1:2], NORM)
            m2 = bnp.tile([128, 1], F32, tag="m2")
            nc.scalar.activation(m2[:], mean[:], AF.Square)
            var = bnp.tile([128, 1], F32, tag="var")
            nc.vector.tensor_tensor(var[:], ex2[:], m2[:], SUB)
            std = bnp.tile([128, 1], F32, tag="std")
            nc.scalar.activation(std[:], var[:], AF.Sqrt, bias=eps_sb[:])
            rstd = bnp.tile([128, 1], F32, tag="rstd")
            nc.vector.reciprocal(rstd[:], std[:])
            scale = bnp.tile([128, 1], F32, tag="scale")
            nc.vector.tensor_tensor(scale[:], gt_sb[:, t:t + 1], rstd[:], MULT)
            nscale = bnp.tile([128, 1], F32, tag="nscale")
            nc.scalar.mul(nscale[:], scale[:], -1.0)
            shift = bnp.tile([128, 1], F32, tag="shift")
            nc.vector.scalar_tensor_tensor(
                shift[:], mean[:], nscale[:], bt_sb[:, t:t + 1], MULT, ADD
            )

            # ---- normalize + LeakyReLU + store ----
            for s in range(SPC):
                for h in range(NHT):
                    idx = s * NHT + h
                    z = wkp.tile([128, 512], F32, tag="z")
                    nc.scalar.activation(
                        z[:], stage[:, idx * 512:(idx + 1) * 512], AF.Identity,
                        bias=shift[:], scale=scale[:],
                    )
                    f = wkp.tile([128, 512], F32, tag="f")
                    nc.vector.scalar_tensor_tensor(f[:], z[:], 0.1, z[:], MULT, MAX)
                    nc.sync.dma_start(
                        out_e.ap()[s, t * 128:(t + 1) * 128, h * 512:(h + 1) * 512],
                        f[:],
                    )


_NC_CACHE = {}


def _build(reps=1):
    if reps in _NC_CACHE:
        return _NC_CACHE[reps]
    nc = bacc.Bacc("TRN2", target_bir_lowering=False, debug=False,
                   num_devices=N_CORES)
    with tile.TileContext(nc) as tc:
        _emit(tc, reps=reps, use_collective=(reps == 1))
    nc.compile()
    _NC_CACHE[reps] = nc
    return nc


def make_in_maps(x, expert_weight, expert_bias, fc_w, fc_b, bn_gamma, bn_beta):
    x = np.asarray(x, np.float32)
    ew = np.asarray(expert_weight, np.float32)
    # padded input, split into c_in halves: xp[b, c, p, (i, j)]
    xp = np.zeros((B, 2, 128, HP, HP), np.float32)
    xp[:, :, :, 1:H + 1, 1:W + 1] = x.reshape(B, 2, 128, H, W)
    xp = np.ascontiguousarray(xp.reshape(B, 2, 128, FREE_PAD))
    # expert weights in conv-lhsT layout: [t, c, k, p=ci, tap*128 + m=co]
    e = ew.reshape(NEXP, NCT, 128, 2, 128, KH, KH)
    ewt = np.ascontiguousarray(
        e.transpose(1, 3, 0, 4, 5, 6, 2).reshape(NCT, 2, NEXP, 128, 9 * 128)
    )
    fcwt = np.ascontiguousarray(
        np.asarray(fc_w, np.float32).T.reshape(2, 128, NEXP)
    )
    fcb = np.ascontiguousarray(np.asarray(fc_b, np.float32).reshape(NEXP, 1))
    eb = np.ascontiguousarray(np.asarray(expert_bias, np.float32))
    gt = np.ascontiguousarray(np.asarray(bn_gamma, np.float32).reshape(NCT, 128).T)
    bt = np.ascontiguousarray(np.asarray(bn_beta, np.float32).reshape(NCT, 128).T)
    return [
        {
            "xp": np.ascontiguousarray(xp[j * SPC:(j + 1) * SPC]),
            "ew": ewt,
            "eb": eb,
            "fcw": fcwt,
            "fcb": fcb,
            "gt": gt,
            "bt": bt,
        }
        for j in range(N_CORES)
    ]


def kernel(x, expert_weight, expert_bias, fc_w, fc_b, bn_gamma, bn_beta):
    nc = _build()
    in_maps = make_in_maps(x, expert_weight, expert_bias, fc_w, fc_b,
                           bn_gamma, bn_beta)
    res = run_bass_kernel_spmd(nc, in_maps, list(range(N_CORES)))
    return np.concatenate(
        [r["out"].reshape(SPC, COUT, H, W) for r in res.results], axis=0
    ).astype(np.float32)
